# revision 27
# baseline (speedup 1.0000x reference)
"""DynamicEdgeConv layer on Trainium2 (Bass/Tile), data-parallel over batch.

Per core (one batch element, N=4096 points, C=64 channels):
  nd[i,j] = 2*<x_i,x_j> - |x_j|^2            (= -d2[i,j] + |x_i|^2, same row order)
  idx[i,:] = top-16 of nd[i,:]               (exact kNN incl. self, jax tie order)
  h1 = relu(A[i] + Bg[j_k])                  A = xb@(W1a-W1b)+b1, Bg = xb@W1b
  h2 = h1 @ W2                                (bias b2 folded post-max)
  out[:, i] = relu(max_k h2 + b2)

Top-16 uses the DVE max8/max_index/match_replace ucode ops (2 rounds of 8),
whose duplicate-advancing semantics reproduce jax.lax.top_k tie behavior.
The neighbor gather is done with indirect (SWDGE) DMAs reading rows of Bg
staged in DRAM, fused with the +A add via the DMA compute unit.
"""

import numpy as np

import concourse.bacc as bacc
import concourse.bass as bass
import concourse.mybir as mybir
from concourse.bass_utils import run_bass_kernel_spmd
from concourse.masks import make_identity
from concourse.tile import TileContext

F32 = mybir.dt.float32
F32R = mybir.dt.float32r
BF16 = mybir.dt.bfloat16
U32 = mybir.dt.uint32

B, C, N, OUT, K = 8, 64, 4096, 64, 16
P = 128
NT = N // P  # 32 row tiles
JC = 512     # j-chunk per matmul
NJ = N // JC
NEG = -1.0e30

USE_F32R = False      # float32r distance matmul (4x PE) -- only if selection unchanged
TOPK_CHUNKED = False  # per-chunk max8 prefilter before the global rounds
PREFETCH = 3


def _view3(ap2d, mid, inner, mid_step, inner_step):
    """Reinterpret a 2D AP [P, mid*inner] as 3D [P, mid, inner]."""
    a = ap2d.ap
    return bass.AP(
        ap2d.tensor, ap2d.offset, [list(a[0]), [mid_step, mid], [inner_step, inner]]
    )


def build_program():
    nc = bacc.Bacc("TRN2", target_bir_lowering=False, debug=False, num_devices=B)

    x_d = nc.dram_tensor("x", [C, N], F32, kind="ExternalInput")        # xb^T
    x2_d = nc.dram_tensor("x2", [C, N], F32, kind="ExternalInput")      # 2*xb^T
    nsq_d = nc.dram_tensor("nsq", [1, N], F32, kind="ExternalInput")    # -|x_j|^2
    w1aug_d = nc.dram_tensor("w1aug", [C + 1, OUT], F32, kind="ExternalInput")
    w1bh_d = nc.dram_tensor("w1bh", [C, OUT], F32, kind="ExternalInput")
    w2_d = nc.dram_tensor("w2", [C, OUT], F32, kind="ExternalInput")
    b2_d = nc.dram_tensor("b2c", [OUT, 1], F32, kind="ExternalInput")
    out_d = nc.dram_tensor("out", [OUT, N], F32, kind="ExternalOutput")
    bg_d = nc.dram_tensor("bg", [N, OUT], F32, kind="Internal")

    with TileContext(nc) as tc:
        with (
            tc.tile_pool(name="const", bufs=1) as cpool,
            tc.tile_pool(name="nd_sb", bufs=PREFETCH + 1) as ndpool,
            tc.tile_pool(name="mr_sb", bufs=2) as mrpool,
            tc.tile_pool(name="sm_sb", bufs=3) as smpool,
            tc.tile_pool(name="g_sb", bufs=2) as gpool,
            tc.tile_pool(name="ps", bufs=3, space="PSUM") as ndps,
            tc.tile_pool(name="tr_ps", bufs=1, space="PSUM") as trps,
            tc.tile_pool(name="h2_ps", bufs=2, space="PSUM") as h2ps,
        ):
            xnd = cpool.tile([C + 1, N], F32)   # rows 0:64 xb^T, row 64 = -sq
            xa2 = cpool.tile([C + 1, N], F32)   # rows 0:64 2*xb^T, row 64 = ones
            w1aug = cpool.tile([C + 1, OUT], F32)
            w1bh = cpool.tile([C, OUT], F32)
            w2sb = cpool.tile([C, OUT], F32)
            b2sb = cpool.tile([OUT, 1], F32)
            ident = cpool.tile([P, P], F32)
            ones_col = cpool.tile([C, 1], F32)
            a_b = cpool.tile([P, NT * OUT], F32)  # A+b1 for all row tiles

            for q in range(4):
                qsl = slice(q * (N // 4), (q + 1) * (N // 4))
                nc.sync.dma_start(out=xnd[0:C, qsl], in_=x_d[:, qsl])
                nc.sync.dma_start(out=xnd[C : C + 1, qsl], in_=nsq_d[:, qsl])
                nc.scalar.dma_start(out=xa2[0:C, qsl], in_=x2_d[:, qsl])
            nc.gpsimd.memset(xa2[C : C + 1, :], 1.0)
            nc.sync.dma_start(out=w1aug[:], in_=w1aug_d[:])
            nc.sync.dma_start(out=w1bh[:], in_=w1bh_d[:])
            nc.sync.dma_start(out=w2sb[:], in_=w2_d[:])
            nc.sync.dma_start(out=b2sb[:], in_=b2_d[:])
            make_identity(nc, ident[:])
            nc.vector.memset(ones_col[:], 1.0)

            def nd_matmuls(t):
                """Compute nd rows for tile t into a fresh SBUF tile."""
                sl = slice(t * P, (t + 1) * P)
                nd_sb = ndpool.tile([P, N], F32, tag="nd")
                lhs = xa2[:, sl]
                if USE_F32R:
                    lhs = lhs.bitcast(F32R)
                for j in range(NJ):
                    jsl = slice(j * JC, (j + 1) * JC)
                    rhs = xnd[:, jsl]
                    if USE_F32R:
                        rhs = rhs.bitcast(F32R)
                    pnd = ndps.tile([P, JC], F32, tag="pnd")
                    nc.tensor.matmul(out=pnd[:], lhsT=lhs, rhs=rhs, start=True, stop=True)
                    nc.scalar.copy(out=nd_sb[:, jsl], in_=pnd[:])
                return nd_sb

            nd_tiles = {}
            for t in range(PREFETCH):
                nd_tiles[t] = nd_matmuls(t)

            # ---- A+b1 and Bg (staged to DRAM) ----
            for t in range(NT):
                sl = slice(t * P, (t + 1) * P)
                psa = ndps.tile([P, OUT], F32, tag="pnd")
                nc.tensor.matmul(out=psa[:], lhsT=xa2[:, sl], rhs=w1aug[:], start=True, stop=True)
                nc.scalar.copy(out=a_b[:, t * OUT : (t + 1) * OUT], in_=psa[:])
                psb = ndps.tile([P, OUT], F32, tag="pnd")
                nc.tensor.matmul(out=psb[:], lhsT=xa2[0:C, sl], rhs=w1bh[:], start=True, stop=True)
                bgt = smpool.tile([P, OUT], F32, tag="bgt")
                nc.scalar.copy(out=bgt[:], in_=psb[:])
                nc.sync.dma_start(out=bg_d[sl, :], in_=bgt[:])

            # ---- main loop over row tiles ----
            for t in range(NT):
                sl = slice(t * P, (t + 1) * P)
                nd_sb = nd_tiles.pop(t)

                # h1 accumulator prefilled with A_b[i] (gather-adds land on top)
                h1 = gpool.tile([P, K * OUT], F32, tag="h1")
                nc.scalar.copy(
                    out=_view3(h1[:], K, OUT, OUT, 1),
                    in_=_view3(a_b[:, t * OUT : (t + 1) * OUT], K, OUT, 0, 1),
                )

                def gather(k):
                    nc.gpsimd.indirect_dma_start(
                        out=h1[:, k * OUT : (k + 1) * OUT],
                        out_offset=None,
                        in_=bg_d[:],
                        in_offset=bass.IndirectOffsetOnAxis(ap=idx16[:, k : k + 1], axis=0),
                        compute_op=mybir.AluOpType.add,
                    )

                h1t = gpool.tile([OUT, K * P], F32, tag="h1t")
                # both 8-k halves go into one [128, 1024] psum tile: half h on
                # partitions 64h:64h+64, so the k-max reduce uses all 128 lanes
                ph2 = h2ps.tile([P, 8 * P], F32, tag="ph2")

                def mlp_half(h):
                    """Transpose chunks 8h..8h+7, relu-evacuate, W2 matmul."""
                    for c in range(8 * h, 8 * h + 8, 4):
                        ptr = trps.tile([OUT, 4 * P], F32, tag="ptr")
                        for q in range(4):
                            nc.tensor.transpose(
                                out=ptr[:, q * P : (q + 1) * P],
                                in_=h1[:, (c + q) * OUT : (c + q + 1) * OUT],
                                identity=ident[:],
                            )
                        nc.scalar.activation(
                            out=h1t[:, c * P : (c + 4) * P], in_=ptr[:],
                            func=mybir.ActivationFunctionType.Relu,
                        )
                    for q in range(2):
                        csl = slice(h * 8 * P + q * 4 * P, h * 8 * P + (q + 1) * 4 * P)
                        nc.tensor.matmul(
                            out=ph2[64 * h : 64 * h + 64, q * 4 * P : (q + 1) * 4 * P],
                            lhsT=w2sb[:], rhs=h1t[:, csl],
                            start=True, stop=True,
                        )

                # top-16 per row: two rounds of max8; gathers and the first
                # half of the edge MLP overlap the round-2 passes
                v8a = smpool.tile([P, 8], F32, tag="v8a")
                v8b = smpool.tile([P, 8], F32, tag="v8b")
                idx16 = smpool.tile([P, K], U32, tag="idx16")
                nd2 = mrpool.tile([P, N], F32, tag="nd2")
                nc.vector.max(out=v8a[:], in_=nd_sb[:])
                nc.vector.max_index(out=idx16[:, 0:8], in_max=v8a[:], in_values=nd_sb[:])
                for k in range(8):
                    gather(k)
                nc.vector.match_replace(
                    out=nd2[:], in_to_replace=v8a[:], in_values=nd_sb[:], imm_value=NEG
                )
                nc.vector.max(out=v8b[:], in_=nd2[:])
                mlp_half(0)
                nc.vector.max_index(out=idx16[:, 8:16], in_max=v8b[:], in_values=nd2[:])
                for k in range(8, K):
                    gather(k)
                mlp_half(1)

                if t + PREFETCH < NT:
                    nd_tiles[t + PREFETCH] = nd_matmuls(t + PREFETCH)

                # max over k: one full-width reduce, then combine partition halves
                hm128 = smpool.tile([P, P], F32, tag="hm128")
                nc.vector.tensor_reduce(
                    out=hm128[:],
                    in_=_view3(ph2[:], P, 8, 1, P),
                    axis=mybir.AxisListType.X,
                    op=mybir.AluOpType.max,
                )
                hmB = smpool.tile([OUT, P], F32, tag="hmB")
                nc.sync.dma_start(out=hmB[:], in_=hm128[OUT : 2 * OUT, :])
                h2m = smpool.tile([OUT, P], F32, tag="h2m")
                nc.vector.tensor_tensor(
                    out=h2m[:], in0=hm128[0:OUT, :], in1=hmB[:], op=mybir.AluOpType.max
                )
                osb = smpool.tile([OUT, P], F32, tag="osb")
                nc.scalar.activation(
                    out=osb[:], in_=h2m[:],
                    func=mybir.ActivationFunctionType.Relu,
                    bias=b2sb[:], scale=1.0,
                )
                nc.sync.dma_start(out=out_d[:, sl], in_=osb[:])
    nc.compile()
    return nc


_NC_CACHE = None


def _get_program():
    global _NC_CACHE
    if _NC_CACHE is None:
        _NC_CACHE = build_program()
    return _NC_CACHE


def make_in_maps(x, W1, b1, W2, b2):
    x = np.ascontiguousarray(np.asarray(x, np.float32))
    W1 = np.asarray(W1, np.float32)
    b1 = np.asarray(b1, np.float32)
    W2 = np.asarray(W2, np.float32)
    b2 = np.asarray(b2, np.float32)
    w1a, w1b = W1[:C], W1[C:]
    w1aug = np.concatenate([(w1a - w1b) * 0.5, b1[None, :]], axis=0)
    w1bh = w1b * 0.5
    shared = {
        "w1aug": np.ascontiguousarray(w1aug),
        "w1bh": np.ascontiguousarray(w1bh),
        "w2": np.ascontiguousarray(W2),
        "b2c": np.ascontiguousarray(b2[:, None]),
    }
    in_maps = []
    for b in range(B):
        xb = np.ascontiguousarray(x[b, :, :, 0])
        nsq = -np.sum(xb * xb, axis=0, dtype=np.float32)[None, :]
        in_maps.append(
            {
                "x": xb,
                "x2": np.ascontiguousarray(2.0 * xb),
                "nsq": np.ascontiguousarray(nsq),
                **shared,
            }
        )
    return in_maps


def kernel(x, W1, b1, W2, b2):
    nc = _get_program()
    in_maps = make_in_maps(x, W1, b1, W2, b2)
    res = run_bass_kernel_spmd(nc, in_maps, core_ids=list(range(B)))
    out = np.stack([res.results[b]["out"] for b in range(B)], axis=0)
    return out[..., None].astype(np.float32)


if __name__ == "__main__":
    nc = build_program()
    print("program built ok")
